# revision 73
# baseline (speedup 1.0000x reference)
"""BitNet MLP (act_quant -> ternary matmul -> relu^2 -> SubLN -> act_quant ->
ternary matmul) on 8 Trainium2 NeuronCores, data-parallel over tokens.

Math notes (exactness):
- act_quant int levels (|q| <= 127) and ternary weights {-1,0,1} are exactly
  representable in bf16, so both matmuls run on the PE in bf16 with exact
  integer arithmetic (f32 PSUM accumulation, |sums| < 2^24).
- All quantization scales are folded into per-token scalars applied to the
  final [tok, 512] output: out = i2 * beta_t with
    beta_t = clip(c_t * alpha_t^2 * Smax_t * |g0|, 1e-5) * clip(mean|w_dn|,1e-5) / 127
  where alpha_t = clip(max|x_t|,1e-5) * clip(mean|w_up|,1e-5) / 127,
  Smax_t = max_i relu(ih)^2 (bf16-rounded, consistent with the quantized iu),
  c_t = rsqrt(var_t + 1e-6), var_t = alpha_t^4 * sum_i s_i^2 / I.
- Rounding uses the magic-number trick (x + 1.5*2^23 - 1.5*2^23) == RNE
  round-to-integer for |x| < 2^22, matching jnp.round (half-to-even).
- s = relu(ih)^2 is computed in bf16 (<=0.4% element rounding); the same
  bf16 values feed the quantizer, its max (Smax) and the variance sum, so
  quant/dequant scales cancel exactly; residual error ~0.5% max-rel.

Schedule notes:
- Per 128-token tile: U(t) = x-quant + 4 PE transposes + 16 up-matmuls;
  M(t) = relu drain + fused (s=r*r, max s) TTR + quant scale + iu; D(t) =
  16 PE transposes + 16 down-matmuls + PSUM drain. Emission order
  ... M(t), U(t+1), D(t) ... interleaves tile t+1's up-matmuls between
  tile t's up and down PE work so the elementwise chain latency hides.
- s=r*r & max fused in one vector TTR; sum(s^2) on gpsimd STT (accum f32)
  off the critical path; output scale is a batched per-8-tile chain.
"""
import os
import numpy as np

import concourse.bass as bass
import concourse.tile as tile
from concourse import mybir
from concourse.bass_utils import run_bass_kernel_spmd
from concourse.masks import make_identity

# ---------------------------------------------------------------------------
# Workaround for walrus "Too many sync wait commands" on the TileContext tail
# drain: split the drain's semaphore waits across single-wait SP NOPs, then
# advance the observed clocks so the real drain needs none.
import re as _re
import bass_rust as _bass_rust


def _patched_drain_and_barrier(self, tick_clock, wait_clock):
    gc = tick_clock.global_clock
    ticks = list(map(int, _re.findall(r"\d+", repr(gc))))
    n = len(ticks)
    nonzero = [(i, t) for i, t in enumerate(ticks) if t > 0]
    for i, t in nonzero:
        sub = [0] * n
        sub[i] = t
        sub_scoped = _bass_rust.ScopedClock({None: _bass_rust.VectorClock(sub)})
        nop = self.nc.sync.nop()
        wait_clock.add_sem_waits(nop.ins, sub_scoped)
        for ec in wait_clock.engine_clocks:
            ec.update_past(sub_scoped)
    drain_inst = self.nc.sync.drain()
    wait_clock.add_sem_waits(drain_inst.ins,
                             _bass_rust.ScopedClock({None: gc}))
    self.nc.all_engine_barrier()
    popped = self.nc._tile_sem_poison_stack.pop()
    assert popped is self._sem_poison
    self.nc.clear_and_free_semaphores(list(self.sems.allocated().values()))
    self.nc.all_engine_barrier()


tile.TileContext._drain_and_barrier = _patched_drain_and_barrier


def _split_sync_waits(nc, keep_default=1):
    """walrus caps the number of semaphore waits a single instruction can
    carry (CTRL ops take only 1; compute ops a few). Hoist excess waits onto
    single-wait NOPs inserted immediately before the instruction on the same
    engine — identical semantics, engines execute in order."""
    import dataclasses
    keep_by_op = {}
    proto = None
    for f in nc.m.functions:
        for bb in f.blocks:
            for inst in bb.instructions:
                if type(inst).__name__ == "InstNoOp":
                    proto = inst
                    break
            if proto is not None:
                break
        if proto is not None:
            break
    counter = [0]
    for f in nc.m.functions:
        new_blocks = []
        for bb in f.blocks:
            out = []
            changed = False
            for inst in bb.instructions:
                si = inst.sync_info
                ow = list(si.on_wait) if si is not None and si.on_wait else []
                keep = keep_by_op.get(inst.opcode, keep_default)
                if len(ow) > keep:
                    assert proto is not None, "no NoOp prototype found yet"
                    for w in ow[:-keep]:
                        counter[0] += 1
                        nop = dataclasses.replace(
                            proto,
                            name=f"I-waitsplit-{counter[0]}",
                            engine=inst.engine,
                            sync_info=_bass_rust.SyncInfo(on_wait=[w],
                                                          on_update=[]),
                        )
                        out.append(nop)
                    si.on_wait = ow[-keep:]
                    changed = True
                out.append(inst)
            if changed:
                bb2 = _bass_rust.BasicBlock(name=bb.name, instructions=out)
                bb2.IsExit = bb.IsExit
                bb2.IsLoopEntry = bb.IsLoopEntry
                bb2.IsPredicated = bb.IsPredicated
                new_blocks.append(bb2)
            else:
                new_blocks.append(bb)
        f.blocks = new_blocks
# ---------------------------------------------------------------------------

F32 = mybir.dt.float32
BF16 = mybir.dt.bfloat16
ALU = mybir.AluOpType
AF = mybir.ActivationFunctionType

N_CORES = 8
B, S, H, I = 8, 8192, 512, 2048
TOK = B * S                  # 65536 tokens total
TPC = TOK // N_CORES         # 8192 tokens per core
P = 128                      # partition tile
NT = TPC // P                # 64 token tiles per core
NKH = H // P                 # 4 k-tiles over H
NKI = I // P                 # 16 k-tiles over I
NB = I // 512                # 4 psum banks for the up matmul

MAGIC = 12582912.0           # 1.5 * 2^23: RNE round-to-int trick
EPS = 1e-6                   # SubLN eps (from reference)

LAST_RESULT = None           # set by kernel() for test harness introspection


def _emit_weight_quant(nc, tc, consts, ps_pool, ps_tag, ps_shape,
                       wT_dram, n_ktiles, free_len, name, magicb):
    """Quantize a (host-pre-transposed) weight matrix to ternary bf16 tiles.

    Single DMA load (HWDGE) into staged SBUF f32 tiles, then two passes
    over SBUF (abs-sum, then round+clip).  Returns (list of [128, free_len]
    bf16 sbuf tiles, meanclip [128,1] = clip(mean|w|,1e-5) broadcast).
    The staged f32 tiles live in pools scoped to this call.
    """
    from contextlib import ExitStack
    n_elem = n_ktiles * 128 * free_len

    with ExitStack() as ctx:
        stage = ctx.enter_context(tc.tile_pool(name=f"{name}_stage", bufs=1))
        junkp = ctx.enter_context(tc.tile_pool(name=f"{name}_junk", bufs=1))

        wf_tiles = []
        for k in range(n_ktiles):
            wf = stage.tile([P, free_len], F32, tag=f"wf{k}")
            nc.sync.dma_start(out=wf, in_=wT_dram[k * P:(k + 1) * P, :])
            wf_tiles.append(wf)

        # pass 1: per-partition abs sums
        asum = consts.tile([P, n_ktiles], F32, tag=f"{name}_asum")
        junk = junkp.tile([P, free_len], BF16, tag="junk")
        for k in range(n_ktiles):
            nc.scalar.activation(out=junk, in_=wf_tiles[k], func=AF.Abs,
                                 accum_out=asum[:, k:k + 1])
        tot = consts.tile([P, 1], F32, tag=f"{name}_tot")
        nc.vector.tensor_reduce(out=tot, in_=asum, axis=mybir.AxisListType.X,
                                op=ALU.add)
        # broadcast-sum across partitions: ones128.T @ tot into a borrowed
        # slot of a main PSUM pool (prologue-time; ring cycles are free)
        ones128 = junkp.tile([P, P], F32, tag="ones128")
        nc.vector.memset(ones128, 1.0)
        totp = ps_pool.tile(ps_shape, F32, tag=ps_tag, name=f"{name}_totp")
        nc.tensor.matmul(out=totp[:, 0:1], lhsT=ones128, rhs=tot,
                         start=True, stop=True)
        gsum = consts.tile([P, 1], F32, tag=f"{name}_gsum")
        nc.scalar.copy(out=gsum, in_=totp[:, 0:1])
        # mean -> clip -> reciprocal scale
        meanclip = consts.tile([P, 1], F32, tag=f"{name}_meanclip")
        nc.vector.tensor_scalar(out=meanclip, in0=gsum, scalar1=1.0 / n_elem,
                                scalar2=1e-5, op0=ALU.mult, op1=ALU.max)
        swq = consts.tile([P, 1], F32, tag=f"{name}_swq")
        nc.vector.reciprocal(out=swq, in_=meanclip)

        # pass 2: round+clip to ternary bf16 (from the staged SBUF copy)
        wq_tiles = []
        for k in range(n_ktiles):
            rt = junkp.tile([P, free_len], F32, tag="stage_rt", bufs=1)
            nc.scalar.activation(out=rt, in_=wf_tiles[k], func=AF.Identity,
                                 bias=magicb, scale=swq)
            cl = junkp.tile([P, free_len], F32, tag="stage_cl", bufs=1)
            nc.vector.tensor_scalar(out=cl, in0=rt, scalar1=MAGIC,
                                    scalar2=1.0, op0=ALU.subtract,
                                    op1=ALU.min)
            wq = consts.tile([P, free_len], BF16, tag=f"{name}_wq{k}")
            nc.vector.tensor_scalar(out=wq, in0=cl, scalar1=-1.0,
                                    scalar2=None, op0=ALU.max)
            wq_tiles.append(wq)
    return wq_tiles, meanclip


def build_nc(general_g: bool):
    nc = bass.Bass()
    x_d = nc.dram_tensor("x", [TPC, H], F32, kind="ExternalInput")
    wupT_d = nc.dram_tensor("wupT", [H, I], F32, kind="ExternalInput")
    wdnT_d = nc.dram_tensor("wdnT", [I, H], F32, kind="ExternalInput")
    g_d = nc.dram_tensor("g", [I], F32, kind="ExternalInput")
    out_d = nc.dram_tensor("out", [TPC, H], F32, kind="ExternalOutput")

    from contextlib import ExitStack
    with ExitStack() as ctx:
        tc = ctx.enter_context(tile.TileContext(nc))

        # ---------------- constants / weight prep ----------------
        consts = ctx.enter_context(tc.tile_pool(name="consts", bufs=1))

        ident = consts.tile([P, P], BF16)
        make_identity(nc, ident)

        magicb = consts.tile([P, 1], F32)
        nc.vector.memset(magicb, MAGIC)
        nmagicb = consts.tile([P, 1], F32)
        nc.vector.memset(nmagicb, -MAGIC)

        g_bc = None
        if general_g:
            # g broadcast to all partitions: [128, I] f32
            g_bc = consts.tile([P, I], F32)
            g_ap = g_d[:]
            g_bcast_ap = bass.AP(tensor=g_ap.tensor, offset=g_ap.offset,
                                 ap=[[0, P]] + list(g_ap.ap))
            nc.gpsimd.dma_start(out=g_bc, in_=g_bcast_ap)

        g0b = consts.tile([P, 1], F32)
        with ExitStack() as gctx:
            gps = gctx.enter_context(tc.tile_pool(name="gps", bufs=1,
                                                  space="PSUM"))
            gstage = gctx.enter_context(tc.tile_pool(name="gstage", bufs=1))
            # g0 broadcast [128,1] via K=1 matmul with ones
            ones_row = gstage.tile([1, P], F32, tag="ones_row")
            nc.vector.memset(ones_row, 1.0)
            g0_sb = gstage.tile([1, 1], F32, tag="g0sb")
            nc.gpsimd.dma_start(out=g0_sb, in_=g_d[0:1])
            g0_ps = gps.tile([P, 1], F32, tag="g0ps")
            nc.tensor.matmul(out=g0_ps, lhsT=ones_row, rhs=g0_sb, start=True,
                             stop=True)
            nc.scalar.copy(out=g0b, in_=g0_ps)

        # weights are quantized mid-prologue (below); placeholders for the
        # emit closures, assigned before first use.
        wup_q = wdn_q = None
        k1b = consts.tile([P, 1], F32)
        wdk = consts.tile([P, 1], F32)
        isg = consts.tile([P, 1], F32)
        g0a = consts.tile([P, 1], F32)

        def emit_gain_consts(up_meanclip, dn_meanclip):
            nc.vector.tensor_scalar_mul(out=k1b, in0=up_meanclip,
                                        scalar1=1.0 / 127.0)
            nc.vector.tensor_scalar_mul(out=wdk, in0=dn_meanclip,
                                        scalar1=1.0 / 127.0)
            sg127 = consts.tile([P, 1], F32)
            nc.scalar.activation(out=sg127, in_=g0b, func=AF.Sign)
            nc.vector.tensor_scalar_mul(out=sg127, in0=sg127, scalar1=127.0)
            nc.scalar.activation(out=g0a, in_=g0b, func=AF.Abs)
            # isg folds the quant scale sign so
            #   dr = recip(max(Smax,1e-30) * isg) = sign*127/Smax  (const g)
            if general_g:
                nc.vector.memset(isg, 1.0 / 127.0)
            else:
                nc.vector.tensor_scalar_mul(out=isg, in0=sg127,
                                            scalar1=1.0 / (127.0 * 127.0))

        # ---------------- main token-tile pipeline ----------------
        BG = 8  # tiles per stats batch

        xs_pool = ctx.enter_context(tc.tile_pool(name="xs", bufs=13))
        xq_pool = ctx.enter_context(tc.tile_pool(name="xqp", bufs=4))
        rp = ctx.enter_context(tc.tile_pool(name="rp", bufs=2))
        sp = ctx.enter_context(tc.tile_pool(name="sp", bufs=2))
        rtp = ctx.enter_context(tc.tile_pool(name="rtp", bufs=2))
        iup = ctx.enter_context(tc.tile_pool(name="iup", bufs=3))
        outp = ctx.enter_context(tc.tile_pool(name="outp", bufs=BG + 1))
        o2p = ctx.enter_context(tc.tile_pool(name="o2p", bufs=3))
        junkp = ctx.enter_context(tc.tile_pool(name="mjunk", bufs=1))
        small = ctx.enter_context(tc.tile_pool(name="small", bufs=3))
        batchp = ctx.enter_context(tc.tile_pool(name="batchp", bufs=4))
        # PSUM budget (8 banks): xT 1, ih quarters 4, iuT 1, o 2
        ps_xT = ctx.enter_context(tc.tile_pool(name="ps_xT", bufs=1,
                                               space="PSUM"))
        ps_ih = ctx.enter_context(tc.tile_pool(name="ps_ih", bufs=4,
                                               space="PSUM"))
        ps_iuT = ctx.enter_context(tc.tile_pool(name="ps_iuT", bufs=1,
                                                space="PSUM"))
        ps_o = ctx.enter_context(tc.tile_pool(name="ps_o", bufs=2,
                                              space="PSUM"))

        IH4 = I // 4  # up matmul accumulates one psum bank at a time

        KV = (1.0 / I) if general_g else (1.0 / (127.0 * 127.0 * I))

        batch_state = {}   # ib -> dict of batch stat tiles
        tile_state = {}    # t -> dict of live tiles
        c_state = {}       # ib -> b8 output-scale tile

        def emit_A(ib):
            """Prefetch batch ib: 8 x-tile DMAs + absmax, batched scale chain."""
            xm8 = batchp.tile([P, BG], F32, tag="xm8")
            x_tiles = []
            for j in range(BG):
                r0 = (ib + j) * P
                x_sb = xs_pool.tile([P, H], F32, tag="x")
                nc.gpsimd.dma_start(out=x_sb, in_=x_d[r0:r0 + P, :])
                x_tiles.append(x_sb)
                nc.vector.tensor_reduce(out=xm8[:, j:j + 1], in_=x_sb,
                                        axis=mybir.AxisListType.X, op=ALU.max,
                                        apply_absolute_value=True)
            t08 = batchp.tile([P, BG], F32, tag="t08")
            nc.vector.tensor_scalar_max(out=t08, in0=xm8, scalar1=1e-5)
            xr8 = batchp.tile([P, BG], F32, tag="xr8")
            nc.vector.reciprocal(out=xr8, in_=t08)
            xsc8 = batchp.tile([P, BG], F32, tag="xsc8")
            nc.vector.tensor_scalar_mul(out=xsc8, in0=xr8, scalar1=127.0)
            Sm8 = batchp.tile([P, BG], F32, tag="Sm8")
            q28 = batchp.tile([P, BG], F32, tag="q28")
            batch_state[ib] = dict(x_tiles=x_tiles, t08=t08, xsc8=xsc8,
                                   Sm8=Sm8, q28=q28)

        def emit_U1a(t):
            """x-quant for tile t (ACT + DVE), 4 tiles ahead."""
            ib = (t // BG) * BG
            j = t - ib
            bs = batch_state[ib]
            x_sb = bs["x_tiles"][j]
            xq = xq_pool.tile([P, H], F32, tag="xq", bufs=2)
            nc.scalar.activation(out=xq, in_=x_sb, func=AF.Identity,
                                 bias=magicb, scale=bs["xsc8"][:, j:j + 1])
            ix = xq_pool.tile([P, H], BF16, tag="ix", bufs=3)
            nc.scalar.activation(out=ix, in_=xq, func=AF.Identity,
                                 bias=nmagicb)
            tile_state[("ix", t)] = ix

        def emit_U1b(t):
            """PE transposes of ix + ACT drain for tile t, 3 tiles ahead."""
            ix = tile_state.pop(("ix", t))
            xT_ps = ps_xT.tile([P, NKH, P], BF16, tag="xT")
            for k in range(NKH):
                nc.tensor.transpose(out=xT_ps[:, k, :],
                                    in_=ix[:, k * P:(k + 1) * P],
                                    identity=ident)
            xT_sb = xq_pool.tile([P, NKH, P], BF16, tag="xTsb")
            nc.vector.tensor_copy(
                xT_sb.rearrange("p a b -> p (a b)"),
                xT_ps.rearrange("p a b -> p (a b)"))
            tile_state[("xT", t)] = xT_sb

        def emit_U2(t):
            """up matmul for tile t, one PSUM bank (512 outputs) at a time."""
            xT_sb = tile_state.pop(("xT", t))
            ih_quarters = []
            for q in range(NB):
                ihq = ps_ih.tile([P, IH4], F32, tag="ih")
                for k in range(NKH):
                    nc.tensor.matmul(
                        out=ihq,
                        lhsT=xT_sb[:, k, :],
                        rhs=wup_q[k][:, q * 512:(q + 1) * 512],
                        start=(k == 0), stop=(k == NKH - 1))
                ih_quarters.append(ihq)
            tile_state[t] = dict(ih=ih_quarters)

        def emit_M1(t):
            """relu drain + (const) DMA CCE max-fold of r for tile t."""
            st = tile_state[t]
            ih_quarters = st.pop("ih")

            # relu drain PSUM -> f32 SBUF (exact: ih values are integers)
            r_sb = rp.tile([P, I], F32, tag="r")
            for q in range(NB):
                nc.scalar.activation(out=r_sb[:, q * IH4:(q + 1) * IH4],
                                     in_=ih_quarters[q], func=AF.Relu)
            st["r"] = r_sb

        def emit_M3(t):
            """Quant scale + fused scaled-square + iu + sum for tile t (one
            iteration after M1 so no engine FIFO waits on same-iter input)."""
            ib = (t // BG) * BG
            j = t - ib
            bs = batch_state[ib]
            st = tile_state[t]
            r_sb = st.pop("r")

            if general_g:
                s_sb = sp.tile([P, I], F32, tag="s")
                nc.vector.tensor_tensor(out=s_sb, in0=r_sb, in1=r_sb,
                                        op=ALU.mult)
                junk2 = junkp.tile([P, I], BF16, tag="junk2")
                nc.scalar.activation(out=junk2, in_=s_sb, func=AF.Square,
                                     accum_out=bs["q28"][:, j:j + 1])
                sg = rtp.tile([P, I], F32, tag="sg")
                nc.vector.tensor_tensor(out=sg, in0=s_sb, in1=g_bc,
                                        op=ALU.mult)
                nc.vector.tensor_reduce(out=bs["Sm8"][:, j:j + 1], in_=sg,
                                        axis=mybir.AxisListType.X, op=ALU.max,
                                        apply_absolute_value=True)
                sc2 = small.tile([P, 1], F32, tag="sc2")
                nc.vector.tensor_scalar(out=sc2, in0=bs["Sm8"][:, j:j + 1],
                                        scalar1=1e-30, scalar2=isg,
                                        op0=ALU.max, op1=ALU.mult)
                dr = small.tile([P, 1], F32, tag="dr")
                nc.vector.reciprocal(out=dr, in_=sc2)
                rt = rtp.tile([P, I], F32, tag="rt")
                nc.vector.tensor_scalar(out=rt, in0=sg, scalar1=dr,
                                        scalar2=MAGIC, op0=ALU.mult,
                                        op1=ALU.add)
                iu = iup.tile([P, I], BF16, tag="iu")
                nc.vector.tensor_scalar(out=iu, in0=rt, scalar1=MAGIC,
                                        scalar2=None, op0=ALU.subtract)
                st["iu"] = iu
                return

            # mr = max(relu(ih)); scc = max(mr^2, 1e-30) (into Sm8 col for C);
            # dr = 127*sign(g0)/scc
            mr = small.tile([P, 1], F32, tag="mr")
            nc.vector.tensor_reduce(out=mr, in_=r_sb,
                                    axis=mybir.AxisListType.X, op=ALU.max)
            nc.vector.tensor_scalar(out=bs["Sm8"][:, j:j + 1], in0=mr,
                                    scalar1=mr, scalar2=1e-30,
                                    op0=ALU.mult, op1=ALU.max)
            sc2 = small.tile([P, 1], F32, tag="sc2")
            nc.vector.tensor_scalar(out=sc2, in0=bs["Sm8"][:, j:j + 1],
                                    scalar1=isg, scalar2=None, op0=ALU.mult)
            dr = small.tile([P, 1], F32, tag="dr")
            nc.vector.reciprocal(out=dr, in_=sc2)

            # s' = (r*dr)*r = relu(ih)^2 * dr in ONE fused STT (exact f32)
            sd = sp.tile([P, I], F32, tag="s")
            nc.vector.scalar_tensor_tensor(out=sd, in0=r_sb, scalar=dr,
                                           in1=r_sb, op0=ALU.mult,
                                           op1=ALU.mult)
            # iu = RNE(s') via magic add+sub, single DVE op
            iu = iup.tile([P, I], BF16, tag="iu")
            nc.vector.tensor_scalar(out=iu, in0=sd, scalar1=MAGIC,
                                    scalar2=MAGIC, op0=ALU.add,
                                    op1=ALU.subtract)
            # q2 = sum(s'^2) = dr^2 * sum(s^2) on ACT (exact f32 accum);
            # the dr^2 factor is removed in the batched C chain.
            junk2 = junkp.tile([P, I], BF16, tag="junk2")
            nc.scalar.activation(out=junk2, in_=sd, func=AF.Square,
                                 accum_out=bs["q28"][:, j:j + 1])
            st["iu"] = iu

        def emit_odrain(t):
            """Drain tile t's down-matmul PSUM to SBUF (lagged one tile so
            the ACT queue never stalls on an in-flight down matmul)."""
            o_ps = tile_state.pop(("ops", t))
            o_sb = outp.tile([P, H], F32, tag="osb")
            nc.scalar.copy(out=o_sb, in_=o_ps)
            tile_state[("o", t)] = o_sb

        def emit_D(t):
            """PE transposes of iu + down matmul for tile t."""
            st = tile_state.pop(t)
            iu = st["iu"]

            iuT_sbs = []
            for hf in range(2):
                iuT_ps = ps_iuT.tile([P, NKI // 2, P], BF16, tag="iuT")
                for k in range(NKI // 2):
                    kk = hf * (NKI // 2) + k
                    nc.tensor.transpose(out=iuT_ps[:, k, :],
                                        in_=iu[:, kk * P:(kk + 1) * P],
                                        identity=ident)
                iuT_sb = iup.tile([P, NKI // 2, P], BF16, tag=f"iuTsb{hf}",
                                  bufs=2)
                if hf == 0:
                    nc.scalar.copy(
                        out=iuT_sb.rearrange("p a b -> p (a b)"),
                        in_=iuT_ps.rearrange("p a b -> p (a b)"))
                else:
                    nc.vector.tensor_copy(
                        iuT_sb.rearrange("p a b -> p (a b)"),
                        iuT_ps.rearrange("p a b -> p (a b)"))
                iuT_sbs.append(iuT_sb)

            o_ps = ps_o.tile([P, H], F32, tag="o")
            for k in range(NKI):
                nc.tensor.matmul(out=o_ps,
                                 lhsT=iuT_sbs[k // (NKI // 2)][:, k % (NKI // 2), :],
                                 rhs=wdn_q[k],
                                 start=(k == 0), stop=(k == NKI - 1))
            tile_state[("ops", t)] = o_ps
            if t > 0:
                emit_odrain(t - 1)

        def emit_C(ib):
            """Batched beta chain + scale + store for tiles ib..ib+BG-1."""
            bs = batch_state.pop(ib)
            t08, Sm8, q28 = bs["t08"], bs["Sm8"], bs["q28"]
            if general_g:
                scc8 = batchp.tile([P, BG], F32, tag="scc8")
                nc.vector.tensor_scalar_max(out=scc8, in0=Sm8, scalar1=1e-30)
            else:
                scc8 = Sm8  # already max(mr^2, 1e-30) from M3
            ga8 = batchp.tile([P, BG], F32, tag="ga8")
            nc.vector.tensor_scalar_mul(out=ga8, in0=t08, scalar1=k1b)
            al8 = batchp.tile([P, BG], F32, tag="al8")
            nc.vector.tensor_tensor(out=al8, in0=ga8, in1=ga8, op=ALU.mult)
            m18 = batchp.tile([P, BG], F32, tag="m18")
            nc.vector.tensor_tensor(out=m18, in0=al8, in1=scc8, op=ALU.mult)
            # var = alpha^4 * sum(s^2) / I.  const-g: q28 = dr^2*sum(s^2)
            # with dr = sign*127/scc, so sum(s^2) = q28*scc^2/127^2.
            v18 = batchp.tile([P, BG], F32, tag="v18")
            al28 = batchp.tile([P, BG], F32, tag="al28")
            nc.vector.tensor_tensor(out=al28, in0=al8, in1=al8, op=ALU.mult)
            if general_g:
                nc.vector.tensor_tensor(out=v18, in0=al28, in1=q28,
                                        op=ALU.mult)
            else:
                ss8 = batchp.tile([P, BG], F32, tag="ss8")
                nc.vector.tensor_tensor(out=ss8, in0=scc8, in1=scc8,
                                        op=ALU.mult)
                qs8 = batchp.tile([P, BG], F32, tag="qs8")
                nc.vector.tensor_tensor(out=qs8, in0=q28, in1=ss8,
                                        op=ALU.mult)
                nc.vector.tensor_tensor(out=v18, in0=al28, in1=qs8,
                                        op=ALU.mult)
            Ve8 = batchp.tile([P, BG], F32, tag="Ve8")
            nc.vector.tensor_scalar(out=Ve8, in0=v18, scalar1=KV,
                                    scalar2=EPS, op0=ALU.mult, op1=ALU.add)
            sq8 = batchp.tile([P, BG], F32, tag="sq8")
            nc.scalar.activation(out=sq8, in_=Ve8, func=AF.Sqrt)
            cr8 = batchp.tile([P, BG], F32, tag="cr8")
            nc.vector.reciprocal(out=cr8, in_=sq8)
            # one Newton step for rsqrt accuracy (ACT sqrt is approximate)
            h18 = batchp.tile([P, BG], F32, tag="h18")
            nc.vector.tensor_tensor(out=h18, in0=cr8, in1=cr8, op=ALU.mult)
            h28 = batchp.tile([P, BG], F32, tag="h28")
            nc.vector.tensor_tensor(out=h28, in0=h18, in1=Ve8, op=ALU.mult)
            h38 = batchp.tile([P, BG], F32, tag="h38")
            nc.vector.tensor_scalar(out=h38, in0=h28, scalar1=-0.5,
                                    scalar2=1.5, op0=ALU.mult, op1=ALU.add)
            c8 = batchp.tile([P, BG], F32, tag="c8")
            nc.vector.tensor_tensor(out=c8, in0=cr8, in1=h38, op=ALU.mult)
            if general_g:
                m1g8 = m18
            else:
                m1g8 = batchp.tile([P, BG], F32, tag="m1g8")
                nc.vector.tensor_scalar_mul(out=m1g8, in0=m18, scalar1=g0a)
            mu8 = batchp.tile([P, BG], F32, tag="mu8")
            nc.vector.tensor_tensor(out=mu8, in0=c8, in1=m1g8, op=ALU.mult)
            b8 = batchp.tile([P, BG], F32, tag="b8")
            nc.vector.tensor_scalar(out=b8, in0=mu8, scalar1=1e-5,
                                    scalar2=wdk, op0=ALU.max, op1=ALU.mult)
            c_state[ib] = b8

        def emit_O(t):
            """Scale + store one tile (spread across iterations so the
            ACT queue never sees an 8-tile burst)."""
            ib = (t // BG) * BG
            j = t - ib
            b8 = c_state[ib]
            o_sb = tile_state.pop(("o", t))
            o2 = o2p.tile([P, H], F32, tag="o2")
            nc.scalar.activation(out=o2, in_=o_sb, func=AF.Identity,
                                 scale=b8[:, j:j + 1])
            nc.sync.dma_start(out=out_d[t * P:(t + 1) * P, :], in_=o2)
            if j == BG - 1:
                del c_state[ib]

        # ---- 6-stage software-pipelined emission ----
        # Iteration t emits (stage, tile):
        #   A(batch t+8) | U1a(t+4) xq/ix | U1b(t+3) ix transposes |
        #   M1(t) relu | U2(t+1) up-matmul | M3(t-1) max/dr/s'/iu |
        #   D(t-2) iuT+down (drains o(t-3)) | C(batch t-10) | O(t-10)
        # Every op's cross-engine inputs were produced in an EARLIER
        # iteration, so no engine FIFO ever stalls on same-iteration work.
        # Weight quantization is interleaved into the prologue so the x
        # chain overlaps the weight DMAs + quant passes.
        wup_q, up_meanclip = _emit_weight_quant(
            nc, tc, consts, ps_o, "o", [P, H], wupT_d, NKH, I, "wup", magicb)
        emit_A(0)
        emit_U1a(0)
        emit_U1a(1)
        emit_U1b(0)
        emit_U1a(2)
        emit_U1b(1)
        emit_U1a(3)
        emit_U1b(2)
        emit_U2(0)
        wdn_q, dn_meanclip = _emit_weight_quant(
            nc, tc, consts, ps_o, "o", [P, H], wdnT_d, NKI, H, "wdn", magicb)
        emit_gain_consts(up_meanclip, dn_meanclip)
        for t in range(NT):
            if t % BG == 0 and t + BG < NT:
                emit_A(t + BG)
            if t + 4 < NT:
                emit_U1a(t + 4)
            if t + 3 < NT:
                emit_U1b(t + 3)
            emit_M1(t)
            if t + 1 < NT:
                emit_U2(t + 1)
            if t >= 1:
                emit_M3(t - 1)
            if t >= 2:
                emit_D(t - 2)
            if t % BG == 2 and t > BG + 1:
                emit_C(t - BG - 2)
            if t >= BG + 2:
                emit_O(t - BG - 2)
        emit_M3(NT - 1)
        emit_D(NT - 2)
        emit_D(NT - 1)
        emit_odrain(NT - 1)
        emit_C(NT - BG)
        for t in range(NT - BG - 2, NT):
            emit_O(t)

    _split_sync_waits(nc)
    return nc


_NC_CACHE = {}


def kernel(x, w_up, w_down, g):
    global LAST_RESULT
    x = np.ascontiguousarray(x, dtype=np.float32)
    w_up = np.ascontiguousarray(w_up, dtype=np.float32)
    w_down = np.ascontiguousarray(w_down, dtype=np.float32)
    g = np.ascontiguousarray(g, dtype=np.float32)

    if abs(float(g[0])) < 1e-30 and np.all(g == g[0]):
        return np.zeros_like(x)

    general = not bool(np.all(g == g[0]))
    key = ("gen" if general else "const")
    if key not in _NC_CACHE:
        _NC_CACHE[key] = build_nc(general)
    nc = _NC_CACHE[key]

    xt = x.reshape(TOK, H)
    wupT = np.ascontiguousarray(w_up.T)    # [H, I]
    wdnT = np.ascontiguousarray(w_down.T)  # [I, H]
    in_maps = [
        {"x": xt[c * TPC:(c + 1) * TPC], "wupT": wupT, "wdnT": wdnT, "g": g}
        for c in range(N_CORES)
    ]
    res = run_bass_kernel_spmd(
        nc, in_maps, list(range(N_CORES)),
        trace=bool(os.environ.get("BASS_TRACE")),
    )
    LAST_RESULT = res
    out = np.concatenate([res.results[c]["out"] for c in range(N_CORES)],
                         axis=0)
    return out.reshape(B, S, H)


# revision 75
# speedup vs baseline: 1.0459x; 1.0459x over previous
"""BitNet MLP (act_quant -> ternary matmul -> relu^2 -> SubLN -> act_quant ->
ternary matmul) on 8 Trainium2 NeuronCores, data-parallel over tokens.

Math notes (exactness):
- act_quant int levels (|q| <= 127) and ternary weights {-1,0,1} are exactly
  representable in bf16, so both matmuls run on the PE in bf16 with exact
  integer arithmetic (f32 PSUM accumulation, |sums| < 2^24).
- All quantization scales are folded into per-token scalars applied to the
  final [tok, 512] output: out = i2 * beta_t with
    beta_t = clip(c_t * alpha_t^2 * Smax_t * |g0|, 1e-5) * clip(mean|w_dn|,1e-5) / 127
  where alpha_t = clip(max|x_t|,1e-5) * clip(mean|w_up|,1e-5) / 127,
  Smax_t = max_i relu(ih)^2 (bf16-rounded, consistent with the quantized iu),
  c_t = rsqrt(var_t + 1e-6), var_t = alpha_t^4 * sum_i s_i^2 / I.
- Rounding uses the magic-number trick (x + 1.5*2^23 - 1.5*2^23) == RNE
  round-to-integer for |x| < 2^22, matching jnp.round (half-to-even).
- The whole intermediate path is exact f32: r = relu(ih) (integers),
  s' = (r*dr)*r = dr*relu(ih)^2 via one fused DVE scalar_tensor_tensor,
  iu = RNE(s') via a single two-op tensor_scalar (+M, -M), and
  sum(s'^2) = dr^2*sum(s^2) via one ACT Square+accum (unscaled in C).

Schedule notes (6-stage software pipeline, one stage per iteration):
  A(b+8) x-DMA+absmax | U1a(t+4) xq/ix | U1b(t+3) ix PE-transposes |
  M1(t) relu drain | U2(t+1) 16 up-matmuls | M3(t-1) max/dr/s'/iu |
  D(t-2) 16 iuT transposes + 16 down-matmuls | C(b-10) batched beta |
  O(t-10) scale+store.
  Every op's cross-engine inputs come from an earlier iteration, so no
  engine FIFO ever stalls on same-iteration work.  GPSIMD does no compute
  (it runs 2-7x slow under DVE SBUF-port contention and stalls the whole
  pipeline); DVE/ACT carry all elementwise work, roughly balanced at
  ~8.5 us/tile each vs ~8.3 us/tile of PE matmul+transpose work.
  Weight quantization is interleaved into the prologue (wup before the
  first x-tiles, wdn after the first up-matmul) to shorten the startup
  serial path.
"""
import os
import numpy as np

import concourse.bass as bass
import concourse.tile as tile
from concourse import mybir
from concourse.bass_utils import run_bass_kernel_spmd
from concourse.masks import make_identity

# ---------------------------------------------------------------------------
# Workaround for walrus "Too many sync wait commands" on the TileContext tail
# drain: split the drain's semaphore waits across single-wait SP NOPs, then
# advance the observed clocks so the real drain needs none.
import re as _re
import bass_rust as _bass_rust


def _patched_drain_and_barrier(self, tick_clock, wait_clock):
    gc = tick_clock.global_clock
    ticks = list(map(int, _re.findall(r"\d+", repr(gc))))
    n = len(ticks)
    nonzero = [(i, t) for i, t in enumerate(ticks) if t > 0]
    for i, t in nonzero:
        sub = [0] * n
        sub[i] = t
        sub_scoped = _bass_rust.ScopedClock({None: _bass_rust.VectorClock(sub)})
        nop = self.nc.sync.nop()
        wait_clock.add_sem_waits(nop.ins, sub_scoped)
        for ec in wait_clock.engine_clocks:
            ec.update_past(sub_scoped)
    drain_inst = self.nc.sync.drain()
    wait_clock.add_sem_waits(drain_inst.ins,
                             _bass_rust.ScopedClock({None: gc}))
    self.nc.all_engine_barrier()
    popped = self.nc._tile_sem_poison_stack.pop()
    assert popped is self._sem_poison
    self.nc.clear_and_free_semaphores(list(self.sems.allocated().values()))
    self.nc.all_engine_barrier()


tile.TileContext._drain_and_barrier = _patched_drain_and_barrier


def _split_sync_waits(nc, keep_default=1):
    """walrus caps the number of semaphore waits a single instruction can
    carry (CTRL ops take only 1; compute ops a few). Hoist excess waits onto
    single-wait NOPs inserted immediately before the instruction on the same
    engine — identical semantics, engines execute in order."""
    import dataclasses
    keep_by_op = {}
    proto = None
    for f in nc.m.functions:
        for bb in f.blocks:
            for inst in bb.instructions:
                if type(inst).__name__ == "InstNoOp":
                    proto = inst
                    break
            if proto is not None:
                break
        if proto is not None:
            break
    counter = [0]
    for f in nc.m.functions:
        new_blocks = []
        for bb in f.blocks:
            out = []
            changed = False
            for inst in bb.instructions:
                si = inst.sync_info
                ow = list(si.on_wait) if si is not None and si.on_wait else []
                keep = keep_by_op.get(inst.opcode, keep_default)
                if len(ow) > keep:
                    assert proto is not None, "no NoOp prototype found yet"
                    for w in ow[:-keep]:
                        counter[0] += 1
                        nop = dataclasses.replace(
                            proto,
                            name=f"I-waitsplit-{counter[0]}",
                            engine=inst.engine,
                            sync_info=_bass_rust.SyncInfo(on_wait=[w],
                                                          on_update=[]),
                        )
                        out.append(nop)
                    si.on_wait = ow[-keep:]
                    changed = True
                out.append(inst)
            if changed:
                bb2 = _bass_rust.BasicBlock(name=bb.name, instructions=out)
                bb2.IsExit = bb.IsExit
                bb2.IsLoopEntry = bb.IsLoopEntry
                bb2.IsPredicated = bb.IsPredicated
                new_blocks.append(bb2)
            else:
                new_blocks.append(bb)
        f.blocks = new_blocks
# ---------------------------------------------------------------------------

F32 = mybir.dt.float32
BF16 = mybir.dt.bfloat16
ALU = mybir.AluOpType
AF = mybir.ActivationFunctionType

N_CORES = 8
B, S, H, I = 8, 8192, 512, 2048
TOK = B * S                  # 65536 tokens total
TPC = TOK // N_CORES         # 8192 tokens per core
P = 128                      # partition tile
NT = TPC // P                # 64 token tiles per core
NKH = H // P                 # 4 k-tiles over H
NKI = I // P                 # 16 k-tiles over I
NB = I // 512                # 4 psum banks for the up matmul

MAGIC = 12582912.0           # 1.5 * 2^23: RNE round-to-int trick
EPS = 1e-6                   # SubLN eps (from reference)

LAST_RESULT = None           # set by kernel() for test harness introspection


def _emit_weight_quant(nc, tc, consts, ps_pool, ps_tag, ps_shape,
                       wT_dram, n_ktiles, free_len, name, magicb):
    """Quantize a (host-pre-transposed) weight matrix to ternary bf16 tiles.

    Single DMA load (HWDGE) into staged SBUF f32 tiles, then two passes
    over SBUF (abs-sum, then round+clip).  Returns (list of [128, free_len]
    bf16 sbuf tiles, meanclip [128,1] = clip(mean|w|,1e-5) broadcast).
    The staged f32 tiles live in pools scoped to this call.
    """
    from contextlib import ExitStack
    n_elem = n_ktiles * 128 * free_len

    with ExitStack() as ctx:
        stage = ctx.enter_context(tc.tile_pool(name=f"{name}_stage", bufs=1))
        junkp = ctx.enter_context(tc.tile_pool(name=f"{name}_junk", bufs=1))

        wf_tiles = []
        for k in range(n_ktiles):
            wf = stage.tile([P, free_len], F32, tag=f"wf{k}")
            nc.sync.dma_start(out=wf, in_=wT_dram[k * P:(k + 1) * P, :])
            wf_tiles.append(wf)

        # pass 1: per-partition abs sums
        asum = consts.tile([P, n_ktiles], F32, tag=f"{name}_asum")
        junk = junkp.tile([P, free_len], BF16, tag="junk")
        for k in range(n_ktiles):
            nc.scalar.activation(out=junk, in_=wf_tiles[k], func=AF.Abs,
                                 accum_out=asum[:, k:k + 1])
        tot = consts.tile([P, 1], F32, tag=f"{name}_tot")
        nc.vector.tensor_reduce(out=tot, in_=asum, axis=mybir.AxisListType.X,
                                op=ALU.add)
        # broadcast-sum across partitions: ones128.T @ tot into a borrowed
        # slot of a main PSUM pool (prologue-time; ring cycles are free)
        ones128 = junkp.tile([P, P], F32, tag="ones128")
        nc.vector.memset(ones128, 1.0)
        totp = ps_pool.tile(ps_shape, F32, tag=ps_tag, name=f"{name}_totp")
        nc.tensor.matmul(out=totp[:, 0:1], lhsT=ones128, rhs=tot,
                         start=True, stop=True)
        gsum = consts.tile([P, 1], F32, tag=f"{name}_gsum")
        nc.scalar.copy(out=gsum, in_=totp[:, 0:1])
        # mean -> clip -> reciprocal scale
        meanclip = consts.tile([P, 1], F32, tag=f"{name}_meanclip")
        nc.vector.tensor_scalar(out=meanclip, in0=gsum, scalar1=1.0 / n_elem,
                                scalar2=1e-5, op0=ALU.mult, op1=ALU.max)
        swq = consts.tile([P, 1], F32, tag=f"{name}_swq")
        nc.vector.reciprocal(out=swq, in_=meanclip)

        # pass 2: round+clip to ternary bf16 (from the staged SBUF copy)
        wq_tiles = []
        for k in range(n_ktiles):
            rt = junkp.tile([P, free_len], F32, tag="stage_rt", bufs=1)
            nc.scalar.activation(out=rt, in_=wf_tiles[k], func=AF.Identity,
                                 bias=magicb, scale=swq)
            cl = junkp.tile([P, free_len], F32, tag="stage_cl", bufs=1)
            nc.vector.tensor_scalar(out=cl, in0=rt, scalar1=MAGIC,
                                    scalar2=1.0, op0=ALU.subtract,
                                    op1=ALU.min)
            wq = consts.tile([P, free_len], BF16, tag=f"{name}_wq{k}")
            nc.vector.tensor_scalar(out=wq, in0=cl, scalar1=-1.0,
                                    scalar2=None, op0=ALU.max)
            wq_tiles.append(wq)
    return wq_tiles, meanclip


def build_nc(general_g: bool):
    nc = bass.Bass()
    x_d = nc.dram_tensor("x", [TPC, H], F32, kind="ExternalInput")
    wupT_d = nc.dram_tensor("wupT", [H, I], F32, kind="ExternalInput")
    wdnT_d = nc.dram_tensor("wdnT", [I, H], F32, kind="ExternalInput")
    g_d = nc.dram_tensor("g", [I], F32, kind="ExternalInput")
    out_d = nc.dram_tensor("out", [TPC, H], F32, kind="ExternalOutput")

    from contextlib import ExitStack
    with ExitStack() as ctx:
        tc = ctx.enter_context(tile.TileContext(nc))

        # ---------------- constants / weight prep ----------------
        consts = ctx.enter_context(tc.tile_pool(name="consts", bufs=1))

        ident = consts.tile([P, P], BF16)
        make_identity(nc, ident)

        magicb = consts.tile([P, 1], F32)
        nc.vector.memset(magicb, MAGIC)
        nmagicb = consts.tile([P, 1], F32)
        nc.vector.memset(nmagicb, -MAGIC)

        g_bc = None
        if general_g:
            # g broadcast to all partitions: [128, I] f32
            g_bc = consts.tile([P, I], F32)
            g_ap = g_d[:]
            g_bcast_ap = bass.AP(tensor=g_ap.tensor, offset=g_ap.offset,
                                 ap=[[0, P]] + list(g_ap.ap))
            nc.gpsimd.dma_start(out=g_bc, in_=g_bcast_ap)

        g0b = consts.tile([P, 1], F32)
        with ExitStack() as gctx:
            gps = gctx.enter_context(tc.tile_pool(name="gps", bufs=1,
                                                  space="PSUM"))
            gstage = gctx.enter_context(tc.tile_pool(name="gstage", bufs=1))
            # g0 broadcast [128,1] via K=1 matmul with ones
            ones_row = gstage.tile([1, P], F32, tag="ones_row")
            nc.vector.memset(ones_row, 1.0)
            g0_sb = gstage.tile([1, 1], F32, tag="g0sb")
            nc.gpsimd.dma_start(out=g0_sb, in_=g_d[0:1])
            g0_ps = gps.tile([P, 1], F32, tag="g0ps")
            nc.tensor.matmul(out=g0_ps, lhsT=ones_row, rhs=g0_sb, start=True,
                             stop=True)
            nc.scalar.copy(out=g0b, in_=g0_ps)

        # weights are quantized mid-prologue (below); placeholders for the
        # emit closures, assigned before first use.
        wup_q = wdn_q = None
        k1b = consts.tile([P, 1], F32)
        wdk = consts.tile([P, 1], F32)
        isg = consts.tile([P, 1], F32)
        g0a = consts.tile([P, 1], F32)

        def emit_gain_consts(up_meanclip, dn_meanclip):
            nc.vector.tensor_scalar_mul(out=k1b, in0=up_meanclip,
                                        scalar1=1.0 / 127.0)
            nc.vector.tensor_scalar_mul(out=wdk, in0=dn_meanclip,
                                        scalar1=1.0 / 127.0)
            sg127 = consts.tile([P, 1], F32)
            nc.scalar.activation(out=sg127, in_=g0b, func=AF.Sign)
            nc.vector.tensor_scalar_mul(out=sg127, in0=sg127, scalar1=127.0)
            nc.scalar.activation(out=g0a, in_=g0b, func=AF.Abs)
            # isg folds the quant scale sign so
            #   dr = recip(max(Smax,1e-30) * isg) = sign*127/Smax  (const g)
            if general_g:
                nc.vector.memset(isg, 1.0 / 127.0)
            else:
                nc.vector.tensor_scalar_mul(out=isg, in0=sg127,
                                            scalar1=1.0 / (127.0 * 127.0))

        # ---------------- main token-tile pipeline ----------------
        BG = 8  # tiles per stats batch

        xs_pool = ctx.enter_context(tc.tile_pool(name="xs", bufs=13))
        xq_pool = ctx.enter_context(tc.tile_pool(name="xqp", bufs=4))
        rp = ctx.enter_context(tc.tile_pool(name="rp", bufs=2))
        sp = ctx.enter_context(tc.tile_pool(name="sp", bufs=2))
        rtp = ctx.enter_context(tc.tile_pool(name="rtp", bufs=2))
        iup = ctx.enter_context(tc.tile_pool(name="iup", bufs=3))
        outp = ctx.enter_context(tc.tile_pool(name="outp", bufs=BG + 1))
        o2p = ctx.enter_context(tc.tile_pool(name="o2p", bufs=3))
        junkp = ctx.enter_context(tc.tile_pool(name="mjunk", bufs=1))
        small = ctx.enter_context(tc.tile_pool(name="small", bufs=3))
        batchp = ctx.enter_context(tc.tile_pool(name="batchp", bufs=4))
        # PSUM budget (8 banks): xT 1, ih quarters 4, iuT 1, o 2
        ps_xT = ctx.enter_context(tc.tile_pool(name="ps_xT", bufs=1,
                                               space="PSUM"))
        ps_ih = ctx.enter_context(tc.tile_pool(name="ps_ih", bufs=4,
                                               space="PSUM"))
        ps_iuT = ctx.enter_context(tc.tile_pool(name="ps_iuT", bufs=1,
                                                space="PSUM"))
        ps_o = ctx.enter_context(tc.tile_pool(name="ps_o", bufs=2,
                                              space="PSUM"))

        IH4 = I // 4  # up matmul accumulates one psum bank at a time

        KV = (1.0 / I) if general_g else (1.0 / (127.0 * 127.0 * I))

        batch_state = {}   # ib -> dict of batch stat tiles
        tile_state = {}    # t -> dict of live tiles
        c_state = {}       # ib -> b8 output-scale tile

        def emit_A(ib):
            """Prefetch batch ib: 8 x-tile DMAs + absmax, batched scale chain."""
            xm8 = batchp.tile([P, BG], F32, tag="xm8")
            x_tiles = []
            for j in range(BG):
                r0 = (ib + j) * P
                x_sb = xs_pool.tile([P, H], F32, tag="x")
                nc.sync.dma_start(out=x_sb, in_=x_d[r0:r0 + P, :])
                x_tiles.append(x_sb)
                nc.vector.tensor_reduce(out=xm8[:, j:j + 1], in_=x_sb,
                                        axis=mybir.AxisListType.X, op=ALU.max,
                                        apply_absolute_value=True)
            t08 = batchp.tile([P, BG], F32, tag="t08")
            nc.vector.tensor_scalar_max(out=t08, in0=xm8, scalar1=1e-5)
            xr8 = batchp.tile([P, BG], F32, tag="xr8")
            nc.vector.reciprocal(out=xr8, in_=t08)
            xsc8 = batchp.tile([P, BG], F32, tag="xsc8")
            nc.vector.tensor_scalar_mul(out=xsc8, in0=xr8, scalar1=127.0)
            Sm8 = batchp.tile([P, BG], F32, tag="Sm8")
            q28 = batchp.tile([P, BG], F32, tag="q28")
            batch_state[ib] = dict(x_tiles=x_tiles, t08=t08, xsc8=xsc8,
                                   Sm8=Sm8, q28=q28)

        def emit_U1a(t):
            """x-quant for tile t (ACT + DVE), 4 tiles ahead."""
            ib = (t // BG) * BG
            j = t - ib
            bs = batch_state[ib]
            x_sb = bs["x_tiles"][j]
            xq = xq_pool.tile([P, H], F32, tag="xq", bufs=2)
            nc.scalar.activation(out=xq, in_=x_sb, func=AF.Identity,
                                 bias=magicb, scale=bs["xsc8"][:, j:j + 1])
            ix = xq_pool.tile([P, H], BF16, tag="ix", bufs=3)
            nc.scalar.activation(out=ix, in_=xq, func=AF.Identity,
                                 bias=nmagicb)
            tile_state[("ix", t)] = ix

        def emit_U1b(t):
            """PE transposes of ix + ACT drain for tile t, 3 tiles ahead."""
            ix = tile_state.pop(("ix", t))
            xT_ps = ps_xT.tile([P, NKH, P], BF16, tag="xT")
            for k in range(NKH):
                nc.tensor.transpose(out=xT_ps[:, k, :],
                                    in_=ix[:, k * P:(k + 1) * P],
                                    identity=ident)
            xT_sb = xq_pool.tile([P, NKH, P], BF16, tag="xTsb")
            nc.vector.tensor_copy(
                xT_sb.rearrange("p a b -> p (a b)"),
                xT_ps.rearrange("p a b -> p (a b)"))
            tile_state[("xT", t)] = xT_sb

        def emit_U2(t):
            """up matmul for tile t, one PSUM bank (512 outputs) at a time."""
            xT_sb = tile_state.pop(("xT", t))
            ih_quarters = []
            for q in range(NB):
                ihq = ps_ih.tile([P, IH4], F32, tag="ih")
                for k in range(NKH):
                    nc.tensor.matmul(
                        out=ihq,
                        lhsT=xT_sb[:, k, :],
                        rhs=wup_q[k][:, q * 512:(q + 1) * 512],
                        start=(k == 0), stop=(k == NKH - 1))
                ih_quarters.append(ihq)
            tile_state[t] = dict(ih=ih_quarters)

        def emit_M1(t):
            """relu drain + (const) DMA CCE max-fold of r for tile t."""
            st = tile_state[t]
            ih_quarters = st.pop("ih")

            # relu drain PSUM -> f32 SBUF (exact: ih values are integers)
            r_sb = rp.tile([P, I], F32, tag="r")
            for q in range(NB):
                nc.scalar.activation(out=r_sb[:, q * IH4:(q + 1) * IH4],
                                     in_=ih_quarters[q], func=AF.Relu)
            st["r"] = r_sb

        def emit_M3(t):
            """Quant scale + fused scaled-square + iu + sum for tile t (one
            iteration after M1 so no engine FIFO waits on same-iter input)."""
            ib = (t // BG) * BG
            j = t - ib
            bs = batch_state[ib]
            st = tile_state[t]
            r_sb = st.pop("r")

            if general_g:
                s_sb = sp.tile([P, I], F32, tag="s")
                nc.vector.tensor_tensor(out=s_sb, in0=r_sb, in1=r_sb,
                                        op=ALU.mult)
                junk2 = junkp.tile([P, I], BF16, tag="junk2")
                nc.scalar.activation(out=junk2, in_=s_sb, func=AF.Square,
                                     accum_out=bs["q28"][:, j:j + 1])
                sg = rtp.tile([P, I], F32, tag="sg")
                nc.vector.tensor_tensor(out=sg, in0=s_sb, in1=g_bc,
                                        op=ALU.mult)
                nc.vector.tensor_reduce(out=bs["Sm8"][:, j:j + 1], in_=sg,
                                        axis=mybir.AxisListType.X, op=ALU.max,
                                        apply_absolute_value=True)
                sc2 = small.tile([P, 1], F32, tag="sc2")
                nc.vector.tensor_scalar(out=sc2, in0=bs["Sm8"][:, j:j + 1],
                                        scalar1=1e-30, scalar2=isg,
                                        op0=ALU.max, op1=ALU.mult)
                dr = small.tile([P, 1], F32, tag="dr")
                nc.vector.reciprocal(out=dr, in_=sc2)
                rt = rtp.tile([P, I], F32, tag="rt")
                nc.vector.tensor_scalar(out=rt, in0=sg, scalar1=dr,
                                        scalar2=MAGIC, op0=ALU.mult,
                                        op1=ALU.add)
                iu = iup.tile([P, I], BF16, tag="iu")
                nc.vector.tensor_scalar(out=iu, in0=rt, scalar1=MAGIC,
                                        scalar2=None, op0=ALU.subtract)
                st["iu"] = iu
                return

            # mr = max(relu(ih)); scc = max(mr^2, 1e-30) (into Sm8 col for C);
            # dr = 127*sign(g0)/scc
            mr = small.tile([P, 1], F32, tag="mr")
            nc.vector.tensor_reduce(out=mr, in_=r_sb,
                                    axis=mybir.AxisListType.X, op=ALU.max)
            nc.vector.tensor_scalar(out=bs["Sm8"][:, j:j + 1], in0=mr,
                                    scalar1=mr, scalar2=1e-30,
                                    op0=ALU.mult, op1=ALU.max)
            sc2 = small.tile([P, 1], F32, tag="sc2")
            nc.vector.tensor_scalar(out=sc2, in0=bs["Sm8"][:, j:j + 1],
                                    scalar1=isg, scalar2=None, op0=ALU.mult)
            dr = small.tile([P, 1], F32, tag="dr")
            nc.vector.reciprocal(out=dr, in_=sc2)

            # s' = (r*dr)*r = relu(ih)^2 * dr in ONE fused STT (exact f32)
            sd = sp.tile([P, I], F32, tag="s")
            nc.vector.scalar_tensor_tensor(out=sd, in0=r_sb, scalar=dr,
                                           in1=r_sb, op0=ALU.mult,
                                           op1=ALU.mult)
            # iu = RNE(s') via magic add+sub, single DVE op
            iu = iup.tile([P, I], BF16, tag="iu")
            nc.vector.tensor_scalar(out=iu, in0=sd, scalar1=MAGIC,
                                    scalar2=MAGIC, op0=ALU.add,
                                    op1=ALU.subtract)
            # q2 = sum(s'^2) = dr^2 * sum(s^2) on ACT (exact f32 accum);
            # the dr^2 factor is removed in the batched C chain.
            junk2 = junkp.tile([P, I], BF16, tag="junk2")
            nc.scalar.activation(out=junk2, in_=sd, func=AF.Square,
                                 accum_out=bs["q28"][:, j:j + 1])
            st["iu"] = iu

        def emit_odrain(t):
            """Drain tile t's down-matmul PSUM to SBUF (lagged one tile so
            the ACT queue never stalls on an in-flight down matmul)."""
            o_ps = tile_state.pop(("ops", t))
            o_sb = outp.tile([P, H], F32, tag="osb")
            nc.scalar.copy(out=o_sb, in_=o_ps)
            tile_state[("o", t)] = o_sb

        def emit_D(t):
            """PE transposes of iu + down matmul for tile t."""
            st = tile_state.pop(t)
            iu = st["iu"]

            iuT_sbs = []
            for hf in range(2):
                iuT_ps = ps_iuT.tile([P, NKI // 2, P], BF16, tag="iuT")
                for k in range(NKI // 2):
                    kk = hf * (NKI // 2) + k
                    nc.tensor.transpose(out=iuT_ps[:, k, :],
                                        in_=iu[:, kk * P:(kk + 1) * P],
                                        identity=ident)
                iuT_sb = iup.tile([P, NKI // 2, P], BF16, tag=f"iuTsb{hf}",
                                  bufs=2)
                if hf == 0:
                    nc.scalar.copy(
                        out=iuT_sb.rearrange("p a b -> p (a b)"),
                        in_=iuT_ps.rearrange("p a b -> p (a b)"))
                else:
                    nc.vector.tensor_copy(
                        iuT_sb.rearrange("p a b -> p (a b)"),
                        iuT_ps.rearrange("p a b -> p (a b)"))
                iuT_sbs.append(iuT_sb)

            o_ps = ps_o.tile([P, H], F32, tag="o")
            for k in range(NKI):
                nc.tensor.matmul(out=o_ps,
                                 lhsT=iuT_sbs[k // (NKI // 2)][:, k % (NKI // 2), :],
                                 rhs=wdn_q[k],
                                 start=(k == 0), stop=(k == NKI - 1))
            tile_state[("ops", t)] = o_ps
            if t > 0:
                emit_odrain(t - 1)

        def emit_C(ib):
            """Batched beta chain + scale + store for tiles ib..ib+BG-1."""
            bs = batch_state.pop(ib)
            t08, Sm8, q28 = bs["t08"], bs["Sm8"], bs["q28"]
            if general_g:
                scc8 = batchp.tile([P, BG], F32, tag="scc8")
                nc.vector.tensor_scalar_max(out=scc8, in0=Sm8, scalar1=1e-30)
            else:
                scc8 = Sm8  # already max(mr^2, 1e-30) from M3
            ga8 = batchp.tile([P, BG], F32, tag="ga8")
            nc.vector.tensor_scalar_mul(out=ga8, in0=t08, scalar1=k1b)
            al8 = batchp.tile([P, BG], F32, tag="al8")
            nc.vector.tensor_tensor(out=al8, in0=ga8, in1=ga8, op=ALU.mult)
            m18 = batchp.tile([P, BG], F32, tag="m18")
            nc.vector.tensor_tensor(out=m18, in0=al8, in1=scc8, op=ALU.mult)
            # var = alpha^4 * sum(s^2) / I.  const-g: q28 = dr^2*sum(s^2)
            # with dr = sign*127/scc, so sum(s^2) = q28*scc^2/127^2.
            v18 = batchp.tile([P, BG], F32, tag="v18")
            al28 = batchp.tile([P, BG], F32, tag="al28")
            nc.vector.tensor_tensor(out=al28, in0=al8, in1=al8, op=ALU.mult)
            if general_g:
                nc.vector.tensor_tensor(out=v18, in0=al28, in1=q28,
                                        op=ALU.mult)
            else:
                ss8 = batchp.tile([P, BG], F32, tag="ss8")
                nc.vector.tensor_tensor(out=ss8, in0=scc8, in1=scc8,
                                        op=ALU.mult)
                qs8 = batchp.tile([P, BG], F32, tag="qs8")
                nc.vector.tensor_tensor(out=qs8, in0=q28, in1=ss8,
                                        op=ALU.mult)
                nc.vector.tensor_tensor(out=v18, in0=al28, in1=qs8,
                                        op=ALU.mult)
            Ve8 = batchp.tile([P, BG], F32, tag="Ve8")
            nc.vector.tensor_scalar(out=Ve8, in0=v18, scalar1=KV,
                                    scalar2=EPS, op0=ALU.mult, op1=ALU.add)
            sq8 = batchp.tile([P, BG], F32, tag="sq8")
            nc.scalar.activation(out=sq8, in_=Ve8, func=AF.Sqrt)
            cr8 = batchp.tile([P, BG], F32, tag="cr8")
            nc.vector.reciprocal(out=cr8, in_=sq8)
            # one Newton step for rsqrt accuracy (ACT sqrt is approximate)
            h18 = batchp.tile([P, BG], F32, tag="h18")
            nc.vector.tensor_tensor(out=h18, in0=cr8, in1=cr8, op=ALU.mult)
            h28 = batchp.tile([P, BG], F32, tag="h28")
            nc.vector.tensor_tensor(out=h28, in0=h18, in1=Ve8, op=ALU.mult)
            h38 = batchp.tile([P, BG], F32, tag="h38")
            nc.vector.tensor_scalar(out=h38, in0=h28, scalar1=-0.5,
                                    scalar2=1.5, op0=ALU.mult, op1=ALU.add)
            c8 = batchp.tile([P, BG], F32, tag="c8")
            nc.vector.tensor_tensor(out=c8, in0=cr8, in1=h38, op=ALU.mult)
            if general_g:
                m1g8 = m18
            else:
                m1g8 = batchp.tile([P, BG], F32, tag="m1g8")
                nc.vector.tensor_scalar_mul(out=m1g8, in0=m18, scalar1=g0a)
            mu8 = batchp.tile([P, BG], F32, tag="mu8")
            nc.vector.tensor_tensor(out=mu8, in0=c8, in1=m1g8, op=ALU.mult)
            b8 = batchp.tile([P, BG], F32, tag="b8")
            nc.vector.tensor_scalar(out=b8, in0=mu8, scalar1=1e-5,
                                    scalar2=wdk, op0=ALU.max, op1=ALU.mult)
            c_state[ib] = b8

        def emit_O(t):
            """Scale + store one tile (spread across iterations so the
            ACT queue never sees an 8-tile burst)."""
            ib = (t // BG) * BG
            j = t - ib
            b8 = c_state[ib]
            o_sb = tile_state.pop(("o", t))
            o2 = o2p.tile([P, H], F32, tag="o2")
            nc.scalar.activation(out=o2, in_=o_sb, func=AF.Identity,
                                 scale=b8[:, j:j + 1])
            nc.sync.dma_start(out=out_d[t * P:(t + 1) * P, :], in_=o2)
            if j == BG - 1:
                del c_state[ib]

        # ---- 6-stage software-pipelined emission ----
        # Iteration t emits (stage, tile):
        #   A(batch t+8) | U1a(t+4) xq/ix | U1b(t+3) ix transposes |
        #   M1(t) relu | U2(t+1) up-matmul | M3(t-1) max/dr/s'/iu |
        #   D(t-2) iuT+down (drains o(t-3)) | C(batch t-10) | O(t-10)
        # Every op's cross-engine inputs were produced in an EARLIER
        # iteration, so no engine FIFO ever stalls on same-iteration work.
        # Weight quantization is interleaved into the prologue so the x
        # chain overlaps the weight DMAs + quant passes.
        wup_q, up_meanclip = _emit_weight_quant(
            nc, tc, consts, ps_o, "o", [P, H], wupT_d, NKH, I, "wup", magicb)
        emit_A(0)
        emit_U1a(0)
        emit_U1a(1)
        emit_U1b(0)
        emit_U1a(2)
        emit_U1b(1)
        emit_U1a(3)
        emit_U1b(2)
        emit_U2(0)
        wdn_q, dn_meanclip = _emit_weight_quant(
            nc, tc, consts, ps_o, "o", [P, H], wdnT_d, NKI, H, "wdn", magicb)
        emit_gain_consts(up_meanclip, dn_meanclip)
        for t in range(NT):
            if t % BG == 0 and t + BG < NT:
                emit_A(t + BG)
            if t + 4 < NT:
                emit_U1a(t + 4)
            if t + 3 < NT:
                emit_U1b(t + 3)
            emit_M1(t)
            if t + 1 < NT:
                emit_U2(t + 1)
            if t >= 1:
                emit_M3(t - 1)
            if t >= 2:
                emit_D(t - 2)
            if t % BG == 2 and t > BG + 1:
                emit_C(t - BG - 2)
            if t >= BG + 2:
                emit_O(t - BG - 2)
        emit_M3(NT - 1)
        emit_D(NT - 2)
        emit_D(NT - 1)
        emit_odrain(NT - 1)
        emit_C(NT - BG)
        for t in range(NT - BG - 2, NT):
            emit_O(t)

    _split_sync_waits(nc)
    return nc


_NC_CACHE = {}


def kernel(x, w_up, w_down, g):
    global LAST_RESULT
    x = np.ascontiguousarray(x, dtype=np.float32)
    w_up = np.ascontiguousarray(w_up, dtype=np.float32)
    w_down = np.ascontiguousarray(w_down, dtype=np.float32)
    g = np.ascontiguousarray(g, dtype=np.float32)

    if abs(float(g[0])) < 1e-30 and np.all(g == g[0]):
        return np.zeros_like(x)

    general = not bool(np.all(g == g[0]))
    key = ("gen" if general else "const")
    if key not in _NC_CACHE:
        _NC_CACHE[key] = build_nc(general)
    nc = _NC_CACHE[key]

    xt = x.reshape(TOK, H)
    wupT = np.ascontiguousarray(w_up.T)    # [H, I]
    wdnT = np.ascontiguousarray(w_down.T)  # [I, H]
    in_maps = [
        {"x": xt[c * TPC:(c + 1) * TPC], "wupT": wupT, "wdnT": wdnT, "g": g}
        for c in range(N_CORES)
    ]
    res = run_bass_kernel_spmd(
        nc, in_maps, list(range(N_CORES)),
        trace=bool(os.environ.get("BASS_TRACE")),
    )
    LAST_RESULT = res
    out = np.concatenate([res.results[c]["out"] for c in range(N_CORES)],
                         axis=0)
    return out.reshape(B, S, H)
